# revision 1
# baseline (speedup 1.0000x reference)
"""Trainium2 Bass kernel for nn_AudioDeviceModel (dense_cnn, memory-bound).

The reference model applies a chain of dilated kernel-size-2 convs to a
length-1 sequence with SAME padding.  For dilation d the two taps land at
padded positions 0 and d while the real sample sits at position d//2, so
every conv after the first reduces to its bias; the first conv (dilation 1,
pad_low=0) reduces to tap 0: a dot product of x[b, :] with w1[0, :, 0].
The whole model is therefore

    out[b, j] = (x[b, :] . w1[0, :, 0]) * wd[0, j] + bd_eff[j]
    bd_eff[j] = (b1 + b2 + b3 + b4 + b5) * wd[0, j] + bd[j]

(verified numerically against the jax reference to 1e-7).  This is a pure
memory-bound row-wise dot product over a 512 MiB matrix.

Strategy: data-parallel across 8 NeuronCores (1024 rows each).  Per core,
stream x in natural-layout [128, F] tiles (contiguous per-partition DMA at
full HBM bandwidth) and do multiply+reduce in a single DVE pass per tile
with scalar_tensor_tensor(op0=mult imm 1.0, op1=mult v, accum_out=partial).
The tiny epilogue (outer product with wd plus bias) is one fused
scalar_tensor_tensor per 128-row block.

This container's walrus build only accepts ONE on_wait and ONE on_update
per instruction, while Tile emits multi-wait instructions (kernel-tail
drain, multi-dependency compute ops).  legalize_bir_sync() splits the
extras into standalone EventSemaphore/NoOp instructions on the same engine
(sequencers are in-order, so a wait immediately before an instruction is
equivalent; trailing updates only on non-DMA instructions).
"""

import json

import numpy as np

import concourse.bass as bass
import concourse.mybir as mybir
import concourse.tile as tile
from concourse.bass_utils import run_bass_kernel_spmd

FP32 = mybir.dt.float32

N_CORES = 8
B_FULL = 8192
L = 16384
J = 128
B_CORE = B_FULL // N_CORES  # 1024
P = 128                     # SBUF partitions
F = 8192                    # L-chunk (free dim) per DVE op / DMA tile


def legalize_bir_sync(bir_bytes: bytes) -> bytes:
    """Split >1 on_wait / on_update per instruction for this walrus build."""
    mod = json.loads(bir_bytes)
    for fn in mod["functions"]:
        for bb in fn["blocks"]:
            out = []
            for ins in bb["instructions"]:
                si = ins.get("sync_info")
                waits = (si or {}).get("on_wait") or []
                ups = (si or {}).get("on_update") or []
                if len(waits) > 1:
                    for i, w in enumerate(waits[:-1]):
                        out.append({
                            "debug": ins.get("debug"),
                            "engine": ins["engine"],
                            "ins": [],
                            "outs": [],
                            "name": f"{ins['name']}_lw{i}",
                            "opcode": "EventSemaphore",
                            "sync_info": {"on_update": [], "on_wait": [w]},
                        })
                    si["on_wait"] = [waits[-1]]
                out.append(ins)
                if len(ups) > 1:
                    if ins.get("opcode") == "DMACopy":
                        raise RuntimeError(
                            f"multi-update on DMA {ins['name']} cannot be legalized"
                        )
                    for i, u in enumerate(ups[1:]):
                        out.append({
                            "debug": ins.get("debug"),
                            "engine": ins["engine"],
                            "ins": [],
                            "outs": [],
                            "name": f"{ins['name']}_lu{i}",
                            "opcode": "NoOp",
                            "sync_info": {"on_update": [u], "on_wait": []},
                        })
                    si["on_update"] = [ups[0]]
            bb["instructions"] = out
    return json.dumps(mod).encode()


def install_legalizer(nc):
    orig = nc.to_json_bytes

    def patched():
        return legalize_bir_sync(orig())

    nc.to_json_bytes = patched
    return nc


def build_module(b_core: int = B_CORE, l: int = L, f: int = F) -> bass.Bass:
    n_bb = b_core // P
    n_ch = l // f
    nc = bass.Bass()
    x_ds = [
        nc.dram_tensor(f"x{bb}", [P, l], FP32, kind="ExternalInput")
        for bb in range(n_bb)
    ]
    v_d = nc.dram_tensor("v", [l], FP32, kind="ExternalInput")
    wd_d = nc.dram_tensor("wdrow", [J], FP32, kind="ExternalInput")
    bd_d = nc.dram_tensor("bdeff", [J], FP32, kind="ExternalInput")
    out_d = nc.dram_tensor("out", [b_core, J], FP32, kind="ExternalOutput")

    with tile.TileContext(nc) as tc:
        with (
            tc.tile_pool(name="consts", bufs=1) as consts,
            tc.tile_pool(name="xp", bufs=3) as xp,
            tc.tile_pool(name="accp", bufs=12) as accp,
            tc.tile_pool(name="outp", bufs=2) as outp,
        ):
            # Tiny consts on the gpsimd (SWDGE) ring.
            wd_b = consts.tile([P, J], FP32)
            nc.gpsimd.dma_start(out=wd_b, in_=wd_d[:].unsqueeze(0).partition_broadcast(P))
            bd_b = consts.tile([P, J], FP32)
            nc.gpsimd.dma_start(out=bd_b, in_=bd_d[:].unsqueeze(0).partition_broadcast(P))

            # All in-flight DMAs share the SDMA engines (per-ring FIFO,
            # round-robin across rings), so completions arrive in issue
            # order per ring.  Tiles run in c-MAJOR order: all row-blocks
            # of L-chunk 0, then chunk 1, ...  Each v chunk (own tile, so
            # STTs depend only on it) is needed once per chunk-group and is
            # prefetched one group ahead — the v broadcast cost spreads
            # over the whole run instead of stalling the first row-block.
            rings = (nc.sync, nc.scalar)
            v_bs = [
                consts.tile([P, f], FP32, name=f"vb{c}", tag=f"vb{c}")
                for c in range(n_ch)
            ]
            accs = [
                accp.tile([P, n_ch], FP32, name=f"acc{bb}", tag=f"acc{bb}")
                for bb in range(n_bb)
            ]

            def bcast_v_dma(c):
                # split across both rings so the chunk completes in half the
                # time regardless of which ring is busier
                h = f // 2
                for r in range(2):
                    src = (
                        v_d[c * f + r * h:c * f + (r + 1) * h]
                        .unsqueeze(0)
                        .partition_broadcast(P)
                    )
                    rings[r].dma_start(out=v_bs[c][:, r * h:(r + 1) * h], in_=src)

            # Chunk 0 is needed within ~15us: stride-0 DMA broadcast (4 MiB
            # of ring traffic, unavoidable).  Later chunks have tens of us
            # of slack: broadcast on-chip instead (ones[1,P].T @ v = exact
            # replicate; K=1 so each output is a single product) — costs
            # idle TensorE time plus PSUM->SBUF copies on the DVE, and
            # saves 4 MiB of HBM traffic per core per chunk.
            bcast_v_dma(0)
            mm = 512
            ones = consts.tile([1, P], FP32)
            nc.gpsimd.memset(ones, 1.0)
            vrow = consts.tile([1, l - f], FP32)
            nc.gpsimd.dma_start(out=vrow, in_=v_d[f:].unsqueeze(0))
            pe_copies = []  # deferred PSUM->SBUF copies, interleaved below
            with tc.tile_pool(name="psum", bufs=8, space="PSUM") as psum:
                for c in range(1, n_ch):
                    for k in range((c - 1) * f // mm, c * f // mm):
                        pt = psum.tile([P, mm], FP32)
                        nc.tensor.matmul(
                            pt, ones, vrow[:, k * mm:(k + 1) * mm],
                            start=True, stop=True,
                        )
                        dst = v_bs[c][:, k * mm - (c - 1) * f:
                                      (k + 1) * mm - (c - 1) * f]
                        pe_copies.append((dst, pt))

                for c in range(n_ch):
                    for bb in range(n_bb):
                        x_t = xp.tile([P, f], FP32)
                        rings[bb % 2].dma_start(
                            out=x_t, in_=x_ds[bb][:, c * f:(c + 1) * f]
                        )
                        # drain two pending v copies per tile on the DVE
                        # (their matmuls are long done by the time these run)
                        for _ in range(2):
                            if pe_copies:
                                dst, pt = pe_copies.pop(0)
                                nc.vector.tensor_copy(out=dst, in_=pt)
                        # x_t *= v (in place); acc[:, c] = sum over free dim
                        nc.vector.scalar_tensor_tensor(
                            out=x_t,
                            in0=x_t,
                            scalar=1.0,
                            in1=v_bs[c],
                            op0=mybir.AluOpType.mult,
                            op1=mybir.AluOpType.mult,
                            accum_out=accs[bb][:, c:c + 1],
                        )
                        if c == n_ch - 1:
                            t = accp.tile([P, 1], FP32, name=f"t{bb}", tag="t")
                            nc.vector.tensor_reduce(
                                out=t, in_=accs[bb], axis=mybir.AxisListType.X,
                                op=mybir.AluOpType.add,
                            )
                            # out[b, j] = wd[j] * t[b] + bd_eff[j]
                            o_t = outp.tile([P, J], FP32)
                            nc.vector.scalar_tensor_tensor(
                                out=o_t,
                                in0=wd_b,
                                scalar=t,
                                in1=bd_b,
                                op0=mybir.AluOpType.mult,
                                op1=mybir.AluOpType.add,
                            )
                            nc.gpsimd.dma_start(
                                out=out_d[bb * P:(bb + 1) * P, :], in_=o_t
                            )
    install_legalizer(nc)
    return nc


_module_cache: dict = {}


def get_module() -> bass.Bass:
    if "nc" not in _module_cache:
        _module_cache["nc"] = build_module()
    return _module_cache["nc"]


def make_in_maps(inputs: dict) -> list[dict]:
    """Shard the full inputs into one input map per core (pure data parallel
    on the batch dim; tiny weights replicated)."""
    x = np.ascontiguousarray(np.asarray(inputs["x"], dtype=np.float32))
    w1 = np.asarray(inputs["w1"], dtype=np.float32)
    v = np.ascontiguousarray(w1[0, :, 0])
    s0 = float(sum(
        np.asarray(inputs[k], np.float32).reshape(-1)[0]
        for k in ("b1", "b2", "b3", "b4", "b5")
    ))
    wd_row = np.ascontiguousarray(np.asarray(inputs["wd"], np.float32)[0, :])
    bd = np.asarray(inputs["bd"], np.float32).reshape(-1)
    bd_eff = np.ascontiguousarray((s0 * wd_row + bd).astype(np.float32))

    maps = []
    for c in range(N_CORES):
        m = {"v": v, "wdrow": wd_row, "bdeff": bd_eff}
        base = c * B_CORE
        for bb in range(B_CORE // P):
            m[f"x{bb}"] = np.ascontiguousarray(x[base + bb * P:base + (bb + 1) * P])
        maps.append(m)
    return maps


def kernel(**inputs) -> np.ndarray:
    nc = get_module()
    in_maps = make_in_maps(inputs)
    res = run_bass_kernel_spmd(nc, in_maps, core_ids=list(range(N_CORES)))
    return np.concatenate([r["out"] for r in res.results], axis=0)



# revision 3
# speedup vs baseline: 1.3071x; 1.3071x over previous
"""Trainium2 Bass kernel for nn_AudioDeviceModel (dense_cnn, memory-bound).

The reference model applies a chain of dilated kernel-size-2 convs to a
length-1 sequence with SAME padding.  For dilation d the two taps land at
padded positions 0 and d while the real sample sits at position d//2, so
every conv after the first reduces to its bias; the first conv (dilation 1,
pad_low=0) reduces to tap 0: a dot product of x[b, :] with w1[0, :, 0].
The whole model is therefore

    out[b, j] = (x[b, :] . w1[0, :, 0]) * wd[0, j] + bd_eff[j]
    bd_eff[j] = (b1 + b2 + b3 + b4 + b5) * wd[0, j] + bd[j]

(verified numerically against the jax reference).  This is a pure
memory-bound row-wise dot product over a 512 MiB matrix.

Strategy: data-parallel across 8 NeuronCores (1024 rows each).  Profiling
the fp32 version showed the DMA rings run at ~357 GB/s (the per-core HBM
cap) whenever they have work, so runtime ~= bytes / 357 GB/s + idle gaps.
This version attacks the bytes: x and v are staged to HBM in bf16 (the
dot product is accumulated in fp32 on the DVE, and the epilogue stays
fp32).  Max rel err vs the fp32 reference is ~2e-3, well inside the 2e-2
gate, and HBM traffic halves to 32 MiB/core (~94 us roofline).

- v is broadcast to all 128 partitions ON-CHIP: one 32 KiB DMA loads it
  as [1, L], then TensorE rank-1 matmuls (ones[1,128].T @ v[1,512] ->
  PSUM, exact) with DVE PSUM->SBUF (fp32->bf16, exact) copies replicate
  it.  No stride-0 broadcast DMA; the first instructions on both HWDGE
  rings are x-tile loads.
- x streams in [128, 8192] bf16 (2 MiB) tiles, 6 deep, round-robined
  across both HWDGE rings (sync + scalar); DVE does multiply+reduce per
  tile with scalar_tensor_tensor(accum_out=...) (bf16 streams, fp32
  accumulate).
- row-block-major order: each 128-row block finishes (reduce + fused
  outer-product epilogue + out DMA on the HWDGE rings) while later
  blocks still stream.
- the LAST row-block's L-chunks taper (8192, 4096, 2048, 1024, 512, 512)
  so the final STT after the last DMA byte is ~0.4 us instead of 4.3 us.

This container's walrus build only accepts ONE on_wait and ONE on_update
per instruction, while Tile emits multi-wait instructions (kernel-tail
drain, multi-dependency compute ops).  legalize_bir_sync() splits the
extras into standalone EventSemaphore/NoOp instructions on the same engine
(sequencers are in-order, so a wait immediately before an instruction is
equivalent; trailing updates only on non-DMA instructions).
"""

import json

import ml_dtypes
import numpy as np

import concourse.bass as bass
import concourse.mybir as mybir
import concourse.tile as tile
from concourse.bass_utils import run_bass_kernel_spmd

FP32 = mybir.dt.float32
BF16 = mybir.dt.bfloat16

N_CORES = 8
B_FULL = 8192
L = 16384
J = 128
B_CORE = B_FULL // N_CORES  # 1024
P = 128                     # SBUF partitions
F = 8192                    # main L-chunk (free dim) per DVE op / DMA tile
MM = 512                    # matmul moving free dim (PSUM bank)

# chunk schedule: uniform F for row-blocks 0..6; tapered for the last block
# so the post-last-DMA tail is one short STT instead of a full-size one.
MAIN_CHUNKS = [(c * F, F) for c in range(L // F)]
TAIL_SIZES = [8192, 4096, 2048, 1024, 512, 512]
assert sum(TAIL_SIZES) == L
TAIL_CHUNKS = []
_off = 0
for _s in TAIL_SIZES:
    TAIL_CHUNKS.append((_off, _s))
    _off += _s


def legalize_bir_sync(bir_bytes: bytes) -> bytes:
    """Split >1 on_wait / on_update per instruction for this walrus build."""
    mod = json.loads(bir_bytes)
    for fn in mod["functions"]:
        for bb in fn["blocks"]:
            out = []
            for ins in bb["instructions"]:
                si = ins.get("sync_info")
                waits = (si or {}).get("on_wait") or []
                ups = (si or {}).get("on_update") or []
                if len(waits) > 1:
                    for i, w in enumerate(waits[:-1]):
                        out.append({
                            "debug": ins.get("debug"),
                            "engine": ins["engine"],
                            "ins": [],
                            "outs": [],
                            "name": f"{ins['name']}_lw{i}",
                            "opcode": "EventSemaphore",
                            "sync_info": {"on_update": [], "on_wait": [w]},
                        })
                    si["on_wait"] = [waits[-1]]
                out.append(ins)
                if len(ups) > 1:
                    if ins.get("opcode") == "DMACopy":
                        raise RuntimeError(
                            f"multi-update on DMA {ins['name']} cannot be legalized"
                        )
                    for i, u in enumerate(ups[1:]):
                        out.append({
                            "debug": ins.get("debug"),
                            "engine": ins["engine"],
                            "ins": [],
                            "outs": [],
                            "name": f"{ins['name']}_lu{i}",
                            "opcode": "NoOp",
                            "sync_info": {"on_update": [u], "on_wait": []},
                        })
                    si["on_update"] = [ups[0]]
            bb["instructions"] = out
    return json.dumps(mod).encode()


def install_legalizer(nc):
    orig = nc.to_json_bytes

    def patched():
        return legalize_bir_sync(orig())

    nc.to_json_bytes = patched
    return nc


def build_module(b_core: int = B_CORE, l: int = L) -> bass.Bass:
    n_bb = b_core // P
    n_ch = l // F
    nc = bass.Bass()
    x_ds = [
        nc.dram_tensor(f"x{bb}", [P, l], BF16, kind="ExternalInput")
        for bb in range(n_bb)
    ]
    v_d = nc.dram_tensor("vb", [l], BF16, kind="ExternalInput")
    wd_d = nc.dram_tensor("wdrow", [J], FP32, kind="ExternalInput")
    bd_d = nc.dram_tensor("bdeff", [J], FP32, kind="ExternalInput")
    out_d = nc.dram_tensor("out", [b_core, J], FP32, kind="ExternalOutput")

    with tile.TileContext(nc) as tc:
        with (
            tc.tile_pool(name="consts", bufs=1) as consts,
            tc.tile_pool(name="xp", bufs=6) as xp,
            tc.tile_pool(name="accp", bufs=12) as accp,
            tc.tile_pool(name="outp", bufs=2) as outp,
        ):
            rings = (nc.sync, nc.scalar)

            # v row: the only v HBM traffic (32 KiB), first on the sync ring
            # so it lands right after the preamble.
            vrow = consts.tile([1, l], BF16, name="vrow", tag="vrow")
            nc.sync.dma_start(out=vrow, in_=v_d[:].unsqueeze(0))

            ones = consts.tile([1, P], BF16, name="ones", tag="ones")
            nc.vector.memset(ones, 1.0)

            # tiny epilogue consts on the gpsimd (SWDGE) ring.
            wd_b = consts.tile([P, J], FP32, name="wd_b", tag="wd_b")
            nc.gpsimd.dma_start(
                out=wd_b, in_=wd_d[:].unsqueeze(0).partition_broadcast(P)
            )
            bd_b = consts.tile([P, J], FP32, name="bd_b", tag="bd_b")
            nc.gpsimd.dma_start(
                out=bd_b, in_=bd_d[:].unsqueeze(0).partition_broadcast(P)
            )

            # On-chip broadcast of v to all partitions: ones[1,P].T @ v_slice
            # (K=1 rank-1 product, exact replicate) -> PSUM fp32, then DVE
            # copies (cast back to bf16, exact) PSUM -> SBUF.  Costs idle
            # TensorE + ~19 us of DVE slack; no broadcast DMA traffic.
            v_bs = [
                consts.tile([P, F], BF16, name=f"vb{c}", tag=f"vb{c}")
                for c in range(n_ch)
            ]
            with tc.tile_pool(name="psum", bufs=4, space="PSUM") as psum:
                for g in range(l // (2 * MM)):  # 16 groups of 2 matmuls
                    pt = psum.tile([P, 2 * MM], FP32)
                    for h in range(2):
                        k = (2 * g + h) * MM
                        nc.tensor.matmul(
                            pt[:, h * MM:(h + 1) * MM],
                            ones,
                            vrow[:, k:k + MM],
                            start=True, stop=True,
                        )
                    c, kk = divmod(2 * g * MM, F)
                    nc.vector.tensor_copy(
                        out=v_bs[c][:, kk:kk + 2 * MM], in_=pt
                    )

                # Main stream: row-block-major; x tiles alternate across both
                # HWDGE rings; per-block epilogue + out DMA overlap the rest
                # of the stream.
                ri = 0
                for bb in range(n_bb):
                    chunks = TAIL_CHUNKS if bb == n_bb - 1 else MAIN_CHUNKS
                    acc = accp.tile(
                        [P, len(chunks)], FP32, name=f"acc{bb}", tag=f"acc{bb}"
                    )
                    for ci, (off, f) in enumerate(chunks):
                        x_t = xp.tile([P, F], BF16)
                        rings[ri % 2].dma_start(
                            out=x_t[:, :f], in_=x_ds[bb][:, off:off + f]
                        )
                        ri += 1
                        c0, k0 = divmod(off, F)
                        nc.vector.scalar_tensor_tensor(
                            out=x_t[:, :f],
                            in0=x_t[:, :f],
                            scalar=1.0,
                            in1=v_bs[c0][:, k0:k0 + f],
                            op0=mybir.AluOpType.mult,
                            op1=mybir.AluOpType.mult,
                            accum_out=acc[:, ci:ci + 1],
                        )
                    t = accp.tile([P, 1], FP32, name=f"t{bb}", tag="t")
                    nc.vector.tensor_reduce(
                        out=t, in_=acc, axis=mybir.AxisListType.X,
                        op=mybir.AluOpType.add,
                    )
                    # out[b, j] = wd[j] * t[b] + bd_eff[j]
                    o_t = outp.tile([P, J], FP32)
                    nc.vector.scalar_tensor_tensor(
                        out=o_t,
                        in0=wd_b,
                        scalar=t,
                        in1=bd_b,
                        op0=mybir.AluOpType.mult,
                        op1=mybir.AluOpType.add,
                    )
                    rings[ri % 2].dma_start(
                        out=out_d[bb * P:(bb + 1) * P, :], in_=o_t
                    )
                    ri += 1
    install_legalizer(nc)
    return nc


_module_cache: dict = {}


def get_module() -> bass.Bass:
    if "nc" not in _module_cache:
        _module_cache["nc"] = build_module()
    return _module_cache["nc"]


def make_in_maps(inputs: dict) -> list[dict]:
    """Shard the full inputs into one input map per core (pure data parallel
    on the batch dim; tiny weights replicated).  x and v are staged in bf16
    (fp32 accumulation on device keeps the result inside the error gate)."""
    x = np.asarray(inputs["x"], dtype=np.float32)
    xb = np.ascontiguousarray(x).astype(ml_dtypes.bfloat16)
    w1 = np.asarray(inputs["w1"], dtype=np.float32)
    vb = np.ascontiguousarray(w1[0, :, 0]).astype(ml_dtypes.bfloat16)
    s0 = float(sum(
        np.asarray(inputs[k], np.float32).reshape(-1)[0]
        for k in ("b1", "b2", "b3", "b4", "b5")
    ))
    wd_row = np.ascontiguousarray(np.asarray(inputs["wd"], np.float32)[0, :])
    bd = np.asarray(inputs["bd"], np.float32).reshape(-1)
    bd_eff = np.ascontiguousarray((s0 * wd_row + bd).astype(np.float32))

    maps = []
    for c in range(N_CORES):
        m = {"vb": vb, "wdrow": wd_row, "bdeff": bd_eff}
        base = c * B_CORE
        for bb in range(B_CORE // P):
            m[f"x{bb}"] = np.ascontiguousarray(
                xb[base + bb * P:base + (bb + 1) * P]
            )
        maps.append(m)
    return maps


def kernel(**inputs) -> np.ndarray:
    nc = get_module()
    in_maps = make_in_maps(inputs)
    res = run_bass_kernel_spmd(nc, in_maps, core_ids=list(range(N_CORES)))
    return np.concatenate([r["out"] for r in res.results], axis=0)


# revision 4
# speedup vs baseline: 1.4091x; 1.0780x over previous
"""Trainium2 Bass kernel for nn_AudioDeviceModel (dense_cnn, memory-bound).

The reference model applies a chain of dilated kernel-size-2 convs to a
length-1 sequence with SAME padding.  For dilation d the two taps land at
padded positions 0 and d while the real sample sits at position d//2, so
every conv after the first reduces to its bias; the first conv (dilation 1,
pad_low=0) reduces to tap 0: a dot product of x[b, :] with w1[0, :, 0].
The whole model is therefore

    out[b, j] = (x[b, :] . w1[0, :, 0]) * wd[0, j] + bd_eff[j]
    bd_eff[j] = (b1 + b2 + b3 + b4 + b5) * wd[0, j] + bd[j]

(verified numerically against the jax reference).  This is a pure
memory-bound row-wise dot product over a 512 MiB matrix.

Strategy: data-parallel across 8 NeuronCores (1024 rows each).  x and v
are staged to HBM in bf16 (fp32 accumulation on device; max rel err vs
the fp32 reference ~2e-3, well inside the 2e-2 gate), so HBM traffic is
32 MiB/core and the DMA floor ~94 us at the measured 356 GB/s.

Profiling showed scalar_tensor_tensor runs at 1x even for bf16 (8.7 us
per [128,8192] tile), which made DVE the bottleneck (161 us).  So the
multiply+reduce is split across engines to fit under the DMA floor:

- row-blocks 0 and 4: DVE scalar_tensor_tensor (1x, self-contained).
- the other 6 blocks: DVE tensor_tensor multiply (2x for bf16) writes
  the product in place, then ScalarE activation(Copy, accum_out) does
  the row-sum.  DVE ~85 us, ACT <=91 us, both under the DMA floor.

Because ScalarE now does compute, it issues no DMAs: x tiles alternate
between the sync HWDGE ring and the gpsimd SWDGE ring.  v is broadcast
to all 128 partitions on-chip (TensorE rank-1 matmuls ones.T @ v_slice
-> PSUM, exact; ACT copies PSUM -> SBUF bf16), so the only v HBM
traffic is one 32 KiB row load.  Row-block-major order lets each block
finish (DVE reduce + fused outer-product epilogue) mid-stream; out DMAs
are emitted on the sync ring a few blocks late so their semaphore waits
never stall the ring.  The last row-block's L-chunks taper (8192, 4096,
2048, 1024, 512, 512) to shrink the post-last-byte tail.

This container's walrus build only accepts ONE on_wait and ONE on_update
per instruction, while Tile emits multi-wait instructions (kernel-tail
drain, multi-dependency compute ops).  legalize_bir_sync() splits the
extras into standalone EventSemaphore/NoOp instructions on the same engine
(sequencers are in-order, so a wait immediately before an instruction is
equivalent; trailing updates only on non-DMA instructions).
"""

import json

import ml_dtypes
import numpy as np

import concourse.bass as bass
import concourse.mybir as mybir
import concourse.tile as tile
from concourse.bass_utils import run_bass_kernel_spmd

FP32 = mybir.dt.float32
BF16 = mybir.dt.bfloat16

N_CORES = 8
B_FULL = 8192
L = 16384
J = 128
B_CORE = B_FULL // N_CORES  # 1024
P = 128                     # SBUF partitions
F = 8192                    # main L-chunk (free dim) per DVE op / DMA tile
MM = 512                    # matmul moving free dim (PSUM bank)

STT_BLOCKS = (0, 4)         # row-blocks handled entirely on DVE (1x STT)
OUT_DELAY = 3               # flush out-DMAs this many row-blocks late

MAIN_CHUNKS = [(c * F, F) for c in range(L // F)]
TAIL_SIZES = [8192, 4096, 2048, 1024, 512, 512]
assert sum(TAIL_SIZES) == L
TAIL_CHUNKS = []
_off = 0
for _s in TAIL_SIZES:
    TAIL_CHUNKS.append((_off, _s))
    _off += _s


def legalize_bir_sync(bir_bytes: bytes) -> bytes:
    """Split >1 on_wait / on_update per instruction for this walrus build."""
    mod = json.loads(bir_bytes)
    for fn in mod["functions"]:
        for bb in fn["blocks"]:
            out = []
            for ins in bb["instructions"]:
                si = ins.get("sync_info")
                waits = (si or {}).get("on_wait") or []
                ups = (si or {}).get("on_update") or []
                if len(waits) > 1:
                    for i, w in enumerate(waits[:-1]):
                        out.append({
                            "debug": ins.get("debug"),
                            "engine": ins["engine"],
                            "ins": [],
                            "outs": [],
                            "name": f"{ins['name']}_lw{i}",
                            "opcode": "EventSemaphore",
                            "sync_info": {"on_update": [], "on_wait": [w]},
                        })
                    si["on_wait"] = [waits[-1]]
                out.append(ins)
                if len(ups) > 1:
                    if ins.get("opcode") == "DMACopy":
                        raise RuntimeError(
                            f"multi-update on DMA {ins['name']} cannot be legalized"
                        )
                    for i, u in enumerate(ups[1:]):
                        out.append({
                            "debug": ins.get("debug"),
                            "engine": ins["engine"],
                            "ins": [],
                            "outs": [],
                            "name": f"{ins['name']}_lu{i}",
                            "opcode": "NoOp",
                            "sync_info": {"on_update": [u], "on_wait": []},
                        })
                    si["on_update"] = [ups[0]]
            bb["instructions"] = out
    return json.dumps(mod).encode()


def install_legalizer(nc):
    orig = nc.to_json_bytes

    def patched():
        return legalize_bir_sync(orig())

    nc.to_json_bytes = patched
    return nc


def build_module(b_core: int = B_CORE, l: int = L) -> bass.Bass:
    n_bb = b_core // P
    n_ch = l // F
    nc = bass.Bass()
    x_ds = [
        nc.dram_tensor(f"x{bb}", [P, l], BF16, kind="ExternalInput")
        for bb in range(n_bb)
    ]
    v_d = nc.dram_tensor("vb", [l], BF16, kind="ExternalInput")
    wd_d = nc.dram_tensor("wdrow", [J], FP32, kind="ExternalInput")
    bd_d = nc.dram_tensor("bdeff", [J], FP32, kind="ExternalInput")
    out_d = nc.dram_tensor("out", [b_core, J], FP32, kind="ExternalOutput")

    with tile.TileContext(nc) as tc:
        with (
            tc.tile_pool(name="consts", bufs=1) as consts,
            tc.tile_pool(name="xp", bufs=6) as xp,
            tc.tile_pool(name="accp", bufs=20) as accp,
            tc.tile_pool(name="outp", bufs=5) as outp,
        ):
            rings = (nc.sync, nc.gpsimd)

            # v row: the only v HBM traffic (32 KiB), first on the sync ring
            # so it lands right after the preamble.
            vrow = consts.tile([1, l], BF16, name="vrow", tag="vrow")
            nc.sync.dma_start(out=vrow, in_=v_d[:].unsqueeze(0))

            ones = consts.tile([1, P], BF16, name="ones", tag="ones")
            nc.vector.memset(ones, 1.0)

            # tiny epilogue consts on the gpsimd (SWDGE) ring.
            wd_b = consts.tile([P, J], FP32, name="wd_b", tag="wd_b")
            nc.gpsimd.dma_start(
                out=wd_b, in_=wd_d[:].unsqueeze(0).partition_broadcast(P)
            )
            bd_b = consts.tile([P, J], FP32, name="bd_b", tag="bd_b")
            nc.gpsimd.dma_start(
                out=bd_b, in_=bd_d[:].unsqueeze(0).partition_broadcast(P)
            )

            # On-chip broadcast of v to all partitions: ones[1,P].T @ v_slice
            # (K=1 rank-1 product, exact replicate) -> PSUM fp32, then ACT
            # copies (cast back to bf16, exact) PSUM -> SBUF.
            v_bs = [
                consts.tile([P, F], BF16, name=f"vb{c}", tag=f"vb{c}")
                for c in range(n_ch)
            ]
            with tc.tile_pool(name="psum", bufs=4, space="PSUM") as psum:
                for g in range(l // (2 * MM)):  # 16 groups of 2 matmuls
                    pt = psum.tile([P, 2 * MM], FP32)
                    for h in range(2):
                        k = (2 * g + h) * MM
                        nc.tensor.matmul(
                            pt[:, h * MM:(h + 1) * MM],
                            ones,
                            vrow[:, k:k + MM],
                            start=True, stop=True,
                        )
                    c, kk = divmod(2 * g * MM, F)
                    nc.scalar.activation(
                        out=v_bs[c][:, kk:kk + 2 * MM], in_=pt,
                        func=mybir.ActivationFunctionType.Copy,
                    )

                # Main stream: row-block-major; x tiles alternate between the
                # sync HWDGE ring and the gpsimd SWDGE ring.
                pending_outs = []  # (bb, o_t) awaiting out-DMA emission

                def flush_outs(upto_bb):
                    while pending_outs and pending_outs[0][0] <= upto_bb:
                        obb, o_t = pending_outs.pop(0)
                        nc.sync.dma_start(
                            out=out_d[obb * P:(obb + 1) * P, :], in_=o_t
                        )

                ri = 0
                for bb in range(n_bb):
                    chunks = TAIL_CHUNKS if bb == n_bb - 1 else MAIN_CHUNKS
                    acc = accp.tile(
                        [P, len(chunks)], FP32, name=f"acc{bb}", tag=f"acc{bb}"
                    )
                    for ci, (off, f) in enumerate(chunks):
                        flush_outs(bb - OUT_DELAY)
                        x_t = xp.tile([P, F], BF16)
                        rings[ri % 2].dma_start(
                            out=x_t[:, :f], in_=x_ds[bb][:, off:off + f]
                        )
                        ri += 1
                        c0, k0 = divmod(off, F)
                        v_sl = v_bs[c0][:, k0:k0 + f]
                        if bb in STT_BLOCKS:
                            # self-contained 1x multiply+reduce on DVE
                            nc.vector.scalar_tensor_tensor(
                                out=x_t[:, :f],
                                in0=x_t[:, :f],
                                scalar=1.0,
                                in1=v_sl,
                                op0=mybir.AluOpType.mult,
                                op1=mybir.AluOpType.mult,
                                accum_out=acc[:, ci:ci + 1],
                            )
                        else:
                            # 2x multiply on DVE, row-sum on ACT
                            nc.vector.tensor_tensor(
                                out=x_t[:, :f],
                                in0=x_t[:, :f],
                                in1=v_sl,
                                op=mybir.AluOpType.mult,
                            )
                            nc.scalar.activation(
                                out=x_t[:, :f],
                                in_=x_t[:, :f],
                                func=mybir.ActivationFunctionType.Copy,
                                accum_out=acc[:, ci:ci + 1],
                            )
                    t = accp.tile([P, 1], FP32, name=f"t{bb}", tag=f"t{bb}")
                    nc.vector.tensor_reduce(
                        out=t, in_=acc, axis=mybir.AxisListType.X,
                        op=mybir.AluOpType.add,
                    )
                    # out[b, j] = wd[j] * t[b] + bd_eff[j]
                    o_t = outp.tile([P, J], FP32)
                    nc.vector.scalar_tensor_tensor(
                        out=o_t,
                        in0=wd_b,
                        scalar=t,
                        in1=bd_b,
                        op0=mybir.AluOpType.mult,
                        op1=mybir.AluOpType.add,
                    )
                    pending_outs.append((bb, o_t))
                flush_outs(n_bb)
    install_legalizer(nc)
    return nc


_module_cache: dict = {}


def get_module() -> bass.Bass:
    if "nc" not in _module_cache:
        _module_cache["nc"] = build_module()
    return _module_cache["nc"]


def make_in_maps(inputs: dict) -> list[dict]:
    """Shard the full inputs into one input map per core (pure data parallel
    on the batch dim; tiny weights replicated).  x and v are staged in bf16
    (fp32 accumulation on device keeps the result inside the error gate)."""
    x = np.asarray(inputs["x"], dtype=np.float32)
    xb = np.ascontiguousarray(x).astype(ml_dtypes.bfloat16)
    w1 = np.asarray(inputs["w1"], dtype=np.float32)
    vb = np.ascontiguousarray(w1[0, :, 0]).astype(ml_dtypes.bfloat16)
    s0 = float(sum(
        np.asarray(inputs[k], np.float32).reshape(-1)[0]
        for k in ("b1", "b2", "b3", "b4", "b5")
    ))
    wd_row = np.ascontiguousarray(np.asarray(inputs["wd"], np.float32)[0, :])
    bd = np.asarray(inputs["bd"], np.float32).reshape(-1)
    bd_eff = np.ascontiguousarray((s0 * wd_row + bd).astype(np.float32))

    maps = []
    for c in range(N_CORES):
        m = {"vb": vb, "wdrow": wd_row, "bdeff": bd_eff}
        base = c * B_CORE
        for bb in range(B_CORE // P):
            m[f"x{bb}"] = np.ascontiguousarray(
                xb[base + bb * P:base + (bb + 1) * P]
            )
        maps.append(m)
    return maps


def kernel(**inputs) -> np.ndarray:
    nc = get_module()
    in_maps = make_in_maps(inputs)
    res = run_bass_kernel_spmd(nc, in_maps, core_ids=list(range(N_CORES)))
    return np.concatenate([r["out"] for r in res.results], axis=0)


# revision 9
# speedup vs baseline: 1.6193x; 1.1492x over previous
"""Trainium2 Bass kernel for nn_AudioDeviceModel (dense_cnn, memory-bound).

The reference model applies a chain of dilated kernel-size-2 convs to a
length-1 sequence with SAME padding.  For dilation d the two taps land at
padded positions 0 and d while the real sample sits at position d//2, so
every conv after the first reduces to its bias; the first conv (dilation 1,
pad_low=0) reduces to tap 0: a dot product of x[b, :] with w1[0, :, 0].
The whole model is therefore

    out[b, j] = (x[b, :] . w1[0, :, 0]) * wd[0, j] + bd_eff[j]
    bd_eff[j] = (b1 + b2 + b3 + b4 + b5) * wd[0, j] + bd[j]

(verified numerically against the jax reference).  This is a pure
memory-bound row-wise dot product over a 512 MiB matrix.

Strategy: data-parallel across 8 NeuronCores (1024 rows each).  x and v
are staged to HBM in bf16 (fp32 accumulation on device; max rel err vs
the fp32 reference ~2e-3, well inside the 2e-2 gate), so HBM traffic is
32 MiB/core and the DMA floor ~94 us at the measured 356 GB/s.

Profiling showed scalar_tensor_tensor runs at 1x even for bf16 (8.7 us
per [128,8192] tile), which made DVE the bottleneck (161 us).  So the
multiply+reduce is split across engines to fit under the DMA floor:

- row-blocks 0 and 4: DVE scalar_tensor_tensor (1x, self-contained).
- the other 6 blocks: DVE tensor_tensor multiply (2x for bf16) writes
  the product in place, then ScalarE activation(Copy, accum_out) does
  the row-sum.  DVE ~85 us, ACT <=91 us, both under the DMA floor.

Because ScalarE now does compute, it issues no DMAs: x tiles alternate
between the sync HWDGE ring and the gpsimd SWDGE ring.  v is broadcast
to all 128 partitions on-chip (TensorE rank-1 matmuls ones.T @ v_slice
-> PSUM, exact; ACT copies PSUM -> SBUF bf16), so the only v HBM
traffic is one 32 KiB row load.  Row-block-major order lets each block
finish (DVE reduce + fused outer-product epilogue) mid-stream; out DMAs
are emitted on the sync ring a few blocks late so their semaphore waits
never stall the ring.  The last row-block's L-chunks taper (8192, 4096,
2048, 1024, 512, 512) to shrink the post-last-byte tail.

This container's walrus build only accepts ONE on_wait and ONE on_update
per instruction, while Tile emits multi-wait instructions (kernel-tail
drain, multi-dependency compute ops).  legalize_bir_sync() splits the
extras into standalone EventSemaphore/NoOp instructions on the same engine
(sequencers are in-order, so a wait immediately before an instruction is
equivalent; trailing updates only on non-DMA instructions).
"""

import json

import ml_dtypes
import numpy as np

import concourse.bass as bass
import concourse.mybir as mybir
import concourse.tile as tile
from concourse.bass_utils import run_bass_kernel_spmd

FP32 = mybir.dt.float32
BF16 = mybir.dt.bfloat16

N_CORES = 8
B_FULL = 8192
L = 16384
J = 128
B_CORE = B_FULL // N_CORES  # 1024
P = 128                     # SBUF partitions
F = 8192                    # main L-chunk (free dim) per DVE op / DMA tile
MM = 512                    # matmul moving free dim (PSUM bank)

OUT_DELAY = 3               # flush out-DMAs this many row-blocks late

MAIN_CHUNKS = [(c * F, F) for c in range(L // F)]
TAIL_SIZES = [8192, 4096, 2048, 1024, 512, 512]
assert sum(TAIL_SIZES) == L
TAIL_CHUNKS = []
_off = 0
for _s in TAIL_SIZES:
    TAIL_CHUNKS.append((_off, _s))
    _off += _s


def chunk_mode(bb: int, ci: int, f: int) -> str:
    """Assign each chunk to DVE-only STT (1x) or DVE-TT (2x) + ACT accum.

    Balance: DVE cycles = stt + (total - stt)/2, ACT cycles = total - stt
    (ACT activation measured 1x).  Spreading every 4th full chunk to STT
    plus the small taper chunks puts DVE ~86K cycles (~90 us) and ACT
    ~90K cycles (~75 us + 15 us of v-copies), both under the ~96 us DMA
    floor, with no convoy at the tail (the last chunks are short STTs).
    """
    if f < F:
        return "stt"            # taper chunks: short, keep the tail on DVE
    j = 2 * bb + ci             # full-size chunk index 0..13
    return "stt" if j % 4 == 0 else "tt"


def legalize_bir_sync(bir_bytes: bytes) -> bytes:
    """Split >1 on_wait / on_update per instruction for this walrus build."""
    mod = json.loads(bir_bytes)
    for fn in mod["functions"]:
        for bb in fn["blocks"]:
            out = []
            for ins in bb["instructions"]:
                si = ins.get("sync_info")
                waits = (si or {}).get("on_wait") or []
                ups = (si or {}).get("on_update") or []
                if len(waits) > 1:
                    for i, w in enumerate(waits[:-1]):
                        out.append({
                            "debug": ins.get("debug"),
                            "engine": ins["engine"],
                            "ins": [],
                            "outs": [],
                            "name": f"{ins['name']}_lw{i}",
                            "opcode": "EventSemaphore",
                            "sync_info": {"on_update": [], "on_wait": [w]},
                        })
                    si["on_wait"] = [waits[-1]]
                out.append(ins)
                if len(ups) > 1:
                    if ins.get("opcode") == "DMACopy":
                        raise RuntimeError(
                            f"multi-update on DMA {ins['name']} cannot be legalized"
                        )
                    for i, u in enumerate(ups[1:]):
                        out.append({
                            "debug": ins.get("debug"),
                            "engine": ins["engine"],
                            "ins": [],
                            "outs": [],
                            "name": f"{ins['name']}_lu{i}",
                            "opcode": "NoOp",
                            "sync_info": {"on_update": [u], "on_wait": []},
                        })
                    si["on_update"] = [ups[0]]
            bb["instructions"] = out
    return json.dumps(mod).encode()


def install_legalizer(nc):
    orig = nc.to_json_bytes

    def patched():
        return legalize_bir_sync(orig())

    nc.to_json_bytes = patched
    return nc


def build_module(b_core: int = B_CORE, l: int = L) -> bass.Bass:
    n_bb = b_core // P
    n_ch = l // F
    nc = bass.Bass()
    x_ds = [
        nc.dram_tensor(f"x{bb}", [P, l], BF16, kind="ExternalInput")
        for bb in range(n_bb)
    ]
    v_d = nc.dram_tensor("vb", [l], BF16, kind="ExternalInput")
    wd_d = nc.dram_tensor("wdrow", [J], FP32, kind="ExternalInput")
    bd_d = nc.dram_tensor("bdeff", [J], FP32, kind="ExternalInput")
    out_d = nc.dram_tensor("out", [b_core, J], FP32, kind="ExternalOutput")

    with tile.TileContext(nc) as tc:
        with (
            tc.tile_pool(name="consts", bufs=1) as consts,
            tc.tile_pool(name="xp", bufs=8) as xp,
            tc.tile_pool(name="accp", bufs=20) as accp,
            tc.tile_pool(name="outp", bufs=5) as outp,
        ):
            rings = (nc.sync, nc.scalar)

            # v row: the only v HBM traffic (32 KiB), first on the sync ring
            # so it lands right after the preamble.
            vrow = consts.tile([1, l], BF16, name="vrow", tag="vrow")
            nc.sync.dma_start(out=vrow, in_=v_d[:].unsqueeze(0))

            ones = consts.tile([1, P], BF16, name="ones", tag="ones")
            nc.vector.memset(ones, 1.0)

            # tiny epilogue consts on the gpsimd (SWDGE) ring.
            wd_b = consts.tile([P, J], FP32, name="wd_b", tag="wd_b")
            nc.gpsimd.dma_start(
                out=wd_b, in_=wd_d[:].unsqueeze(0).partition_broadcast(P)
            )
            bd_b = consts.tile([P, J], FP32, name="bd_b", tag="bd_b")
            nc.gpsimd.dma_start(
                out=bd_b, in_=bd_d[:].unsqueeze(0).partition_broadcast(P)
            )

            # On-chip broadcast of v to all partitions: ones[1,P].T @ v_slice
            # (K=1 rank-1 product, exact replicate) -> PSUM fp32, then ACT
            # copies (cast back to bf16, exact) PSUM -> SBUF.
            v_bs = [
                consts.tile([P, F], BF16, name=f"vb{c}", tag=f"vb{c}")
                for c in range(n_ch)
            ]
            with tc.tile_pool(name="psum", bufs=2, space="PSUM") as psum:
                for g in range(l // (4 * MM)):  # 8 groups of 4 matmuls
                    pt = psum.tile([P, 4 * MM], FP32)
                    for h in range(4):
                        k = (4 * g + h) * MM
                        nc.tensor.matmul(
                            pt[:, h * MM:(h + 1) * MM],
                            ones,
                            vrow[:, k:k + MM],
                            start=True, stop=True,
                        )
                    c, kk = divmod(4 * g * MM, F)
                    nc.scalar.activation(
                        out=v_bs[c][:, kk:kk + 4 * MM], in_=pt,
                        func=mybir.ActivationFunctionType.Copy,
                    )

                # Main stream: row-block-major; x tiles alternate between the
                # sync HWDGE ring and the gpsimd SWDGE ring.
                pending_outs = []  # (bb, o_t) awaiting out-DMA emission

                def flush_outs(upto_bb):
                    while pending_outs and pending_outs[0][0] <= upto_bb:
                        obb, o_t = pending_outs.pop(0)
                        nc.sync.dma_start(
                            out=out_d[obb * P:(obb + 1) * P, :], in_=o_t
                        )

                ri = 0
                for bb in range(n_bb):
                    chunks = TAIL_CHUNKS if bb == n_bb - 1 else MAIN_CHUNKS
                    acc = accp.tile(
                        [P, len(chunks)], FP32, name=f"acc{bb}", tag=f"acc{bb}"
                    )
                    for ci, (off, f) in enumerate(chunks):
                        flush_outs(bb - OUT_DELAY)
                        x_t = xp.tile([P, F], BF16)
                        rings[ri % 2].dma_start(
                            out=x_t[:, :f], in_=x_ds[bb][:, off:off + f]
                        )
                        ri += 1
                        c0, k0 = divmod(off, F)
                        v_sl = v_bs[c0][:, k0:k0 + f]
                        if chunk_mode(bb, ci, f) == "stt":
                            # self-contained 1x multiply+reduce on DVE
                            nc.vector.scalar_tensor_tensor(
                                out=x_t[:, :f],
                                in0=x_t[:, :f],
                                scalar=1.0,
                                in1=v_sl,
                                op0=mybir.AluOpType.mult,
                                op1=mybir.AluOpType.mult,
                                accum_out=acc[:, ci:ci + 1],
                            )
                        else:
                            # 2x multiply on DVE, row-sum on ACT
                            nc.vector.tensor_tensor(
                                out=x_t[:, :f],
                                in0=x_t[:, :f],
                                in1=v_sl,
                                op=mybir.AluOpType.mult,
                            )
                            nc.scalar.activation(
                                out=x_t[:, :f],
                                in_=x_t[:, :f],
                                func=mybir.ActivationFunctionType.Copy,
                                accum_out=acc[:, ci:ci + 1],
                            )
                    t = accp.tile([P, 1], FP32, name=f"t{bb}", tag=f"t{bb}")
                    nc.vector.tensor_reduce(
                        out=t, in_=acc, axis=mybir.AxisListType.X,
                        op=mybir.AluOpType.add,
                    )
                    # out[b, j] = wd[j] * t[b] + bd_eff[j]
                    o_t = outp.tile([P, J], FP32)
                    nc.vector.scalar_tensor_tensor(
                        out=o_t,
                        in0=wd_b,
                        scalar=t,
                        in1=bd_b,
                        op0=mybir.AluOpType.mult,
                        op1=mybir.AluOpType.add,
                    )
                    pending_outs.append((bb, o_t))
                flush_outs(n_bb)
    install_legalizer(nc)
    return nc


_module_cache: dict = {}


def get_module() -> bass.Bass:
    if "nc" not in _module_cache:
        _module_cache["nc"] = build_module()
    return _module_cache["nc"]


def make_in_maps(inputs: dict) -> list[dict]:
    """Shard the full inputs into one input map per core (pure data parallel
    on the batch dim; tiny weights replicated).  x and v are staged in bf16
    (fp32 accumulation on device keeps the result inside the error gate)."""
    x = np.asarray(inputs["x"], dtype=np.float32)
    xb = np.ascontiguousarray(x).astype(ml_dtypes.bfloat16)
    w1 = np.asarray(inputs["w1"], dtype=np.float32)
    vb = np.ascontiguousarray(w1[0, :, 0]).astype(ml_dtypes.bfloat16)
    s0 = float(sum(
        np.asarray(inputs[k], np.float32).reshape(-1)[0]
        for k in ("b1", "b2", "b3", "b4", "b5")
    ))
    wd_row = np.ascontiguousarray(np.asarray(inputs["wd"], np.float32)[0, :])
    bd = np.asarray(inputs["bd"], np.float32).reshape(-1)
    bd_eff = np.ascontiguousarray((s0 * wd_row + bd).astype(np.float32))

    maps = []
    for c in range(N_CORES):
        m = {"vb": vb, "wdrow": wd_row, "bdeff": bd_eff}
        base = c * B_CORE
        for bb in range(B_CORE // P):
            m[f"x{bb}"] = np.ascontiguousarray(
                xb[base + bb * P:base + (bb + 1) * P]
            )
        maps.append(m)
    return maps


def kernel(**inputs) -> np.ndarray:
    nc = get_module()
    in_maps = make_in_maps(inputs)
    res = run_bass_kernel_spmd(nc, in_maps, core_ids=list(range(N_CORES)))
    return np.concatenate([r["out"] for r in res.results], axis=0)
